# revision 35
# baseline (speedup 1.0000x reference)
"""Trainium2 Bass kernel for GQA causal attention (dense_transformer).

Module: x:[2,2048,1024] -> fused QKV proj (16 Q heads, 4 KV heads, D=64,
only first 1536 rows of w_qkv used) -> causal GQA attention -> out proj.

Sharding (8 NeuronCores): core c = (batch b=c//4, TP rank r=c%4).
Each core owns batch b, query heads 4r..4r+3 and GQA KV head r.
 - QKV projection column-parallel (per-rank weight slices, host-sliced).
 - Attention fully local (GQA group == rank's 4 query heads + 1 KV head).
 - Output projection row-parallel: per-qblock partials ReduceScatter'd
   across the 4-rank TP group; host concatenates rank outputs.

On-device layout notes:
 - Everything runs in "transposed" [feature, seq] layout so the TensorE
   contractions need no on-device transposes.
 - All matmul operands bf16 (1 cycle/row on the PE at any p-state).
 - Softmax without running max (scores ~ N(0,1) after scale, exp is safe).
 - Rowsum via ones-columns fused into the attn*V matmul (free on PE).
 - Scheduling: single closure-list scheduler keeps the PE FIFO busy at
   all times (the PE clock ramps 0.65->1.2->2.4 GHz with idle resets, so
   gaps cost ~2x beyond their length). Out-proj of block j is emitted as
   filler inside block j+1's attention; QKV proj of block j+1 likewise.
 - attn*V trails scores/exp by 2 k-tiles for softmax latency slack.
 - The last 512 q are split 384+128 so the final ReduceScatter (the only
   exposed collective) covers just 128 columns.
"""

import os
import sys

import numpy as np
import ml_dtypes

if "/opt/trn_rl_repo" not in sys.path:
    sys.path.insert(0, "/opt/trn_rl_repo")

B = 2
S = 2048
LATENT = 1024
H = 16
HK = 4
D = 64
NCORES = 8
TP = 4           # tensor-parallel ranks per batch
QH = H // TP     # query heads per core
DQ = QH * D      # 256 attention features per core
SCALE = 1.0 / 8.0
KT = 128
NKT = S // KT    # 16
LCH = LATENT // 128  # 8 contraction chunks
TRAIL = 2        # attn*V trails scores/exp by this many k-tiles

# (q0, qw) attention/out-proj blocks; phase1 (QKV proj) stays on the 512 grid.
# The last 512 q are split 256+256: block 4's compute hides block 3's
# ReduceScatter, and the only exposed collective is the final 256-wide one.
BLOCKS = [(0, 512), (512, 512), (1024, 512), (1536, 256), (1792, 256)]

_CACHE = {}


def _build():
    import concourse.bacc as bacc
    from concourse import mybir
    from concourse.tile import TileContext

    f32 = mybir.dt.float32
    bf16 = mybir.dt.bfloat16
    Exp = mybir.ActivationFunctionType.Exp

    nc = bacc.Bacc("TRN2", target_bir_lowering=False, num_devices=NCORES)

    x_t = nc.declare_dram_parameter("x_t", [LATENT, S], bf16, isOutput=False)
    wq_t = nc.declare_dram_parameter("wq_t", [128, LCH * DQ], bf16,
                                     isOutput=False)
    wk_d = nc.declare_dram_parameter("wk_d", [128, LCH * 128], bf16,
                                     isOutput=False)
    wv_t = nc.declare_dram_parameter("wv_t", [128, LCH * D], bf16,
                                     isOutput=False)
    wo_t = nc.declare_dram_parameter("wo_t", [128, 2 * LATENT], bf16,
                                     isOutput=False)
    mask = nc.declare_dram_parameter("mask", [128, 4 * 1024], bf16,
                                     isOutput=False)
    eye = nc.declare_dram_parameter("eye", [64, 64], f32, isOutput=False)
    out = nc.declare_dram_parameter("out", [DQ, S], bf16, isOutput=True)

    RG = [[0, 1, 2, 3], [4, 5, 6, 7]]

    with TileContext(nc) as tc:
        with (
            tc.tile_pool(name="const", bufs=1) as cst,
            tc.tile_pool(name="sb", bufs=1) as sb,
            tc.tile_pool(name="ps", bufs=1, space="PSUM") as ps,
            tc.tile_pool(name="dram", bufs=1, space="DRAM") as dram,
        ):
            # ---- constants / weights (order = startup critical path) ----
            # warmup collective FIRST: the first collective pays ~40-90us
            # of CC/mesh init (racing the weight DMAs on the shared rings);
            # start it as early as possible so RS(0) never waits on it.
            wup_in = dram.tile([32, 8], bf16, name="wup_in")
            wup_out = dram.tile([8, 8], bf16, name="wup_out")
            wup_sb = cst.tile([32, 8], bf16)
            nc.gpsimd.memset(wup_sb[:], 0.0)
            nc.gpsimd.dma_start(wup_in[:], wup_sb[:])
            nc.gpsimd.collective_compute(
                "ReduceScatter", mybir.AluOpType.add, replica_groups=RG,
                ins=[wup_in[:].opt()], outs=[wup_out[:].opt()],
            )

            ones_f = cst.tile([128, 64], f32)
            nc.vector.memset(ones_f[:], 1.0)
            # 1/64 matrix for the rowsum partition-broadcast matmuls
            scones = cst.tile([128, 64], bf16)
            nc.vector.memset(scones[:], 1.0 / 64.0)
            # preload the exp table set early (overlaps weight DMAs)
            dummy = cst.tile([128, 8], f32)
            nc.scalar.activation(dummy[:], ones_f[:, :8], Exp)

            # weights arrive pre-packed in SBUF layout [128, ...] so each
            # DMA moves wide contiguous rows.  wq's first contraction chunk
            # loads alone (on the otherwise-idle scalar queue) so the very
            # first matmul can start before the bulk of the weights land.
            wqr = wq_t[:].rearrange("p (l m) -> p l m", l=LCH)
            wq_sb = cst.tile([128, LCH, DQ], bf16)
            nc.scalar.dma_start(wq_sb[:, 0:1, :], wqr[:, 0:1, :])
            nc.scalar.dma_start(wq_sb[:, 1:LCH, :], wqr[:, 1:LCH, :])
            wk_sb = cst.tile([128, LCH, 128], bf16)
            nc.gpsimd.dma_start(
                wk_sb[:], wk_d[:].rearrange("p (l m) -> p l m", l=LCH))
            wv_sb = cst.tile([128, LCH, D], bf16)
            nc.gpsimd.dma_start(
                wv_sb[:], wv_t[:].rearrange("p (l m) -> p l m", l=LCH))
            mask_sb = cst.tile([128, 4 * 1024], bf16)
            nc.scalar.dma_start(mask_sb[:, :2048], mask[:, :2048])
            nc.scalar.dma_start(mask_sb[:, 2048:], mask[:, 2048:])
            # wo/eye on the scalar queue: keeps the gpsimd queue free for
            # block 0's startup-critical x chunks
            wo_sb = cst.tile([128, 2, LATENT], bf16)
            nc.scalar.dma_start(
                wo_sb[:], wo_t[:].rearrange("p (l m) -> p l m", l=2))
            eye_sb = cst.tile([64, 64], f32)
            nc.scalar.dma_start(eye_sb[:], eye[:])

            # ---- persistent activations ----
            qT0 = sb.tile([128, S], bf16)   # heads 0,1 (rows 0:64 / 64:128)
            qT1 = sb.tile([128, S], bf16)   # heads 2,3
            qT_sb = [qT0, qT1]
            kT_sb = sb.tile([128, S], bf16)  # duplicated kT (rows 64:128 copy)
            # v tile t at [:, t, 64:128]; ones at [:, t, 0:64] and
            # [:, t, 128:192].  Head-a OT lhsT = vv[:, t, 64:192] = [v|ones]
            # (attn feats at psum rows 0:64, rowsum at 64:128); head-b OT
            # lhsT = vv[:, t, 0:128] = [ones|v] (rowsum at 0:64).
            vv = sb.tile([128, NKT, 192], bf16)
            nc.vector.memset(vv[:, :, 0:64], 1.0)
            nc.vector.memset(vv[:, :, 128:192], 1.0)

            # per-block ReduceScatter bounce buffers, wide rows (16 latent
            # feats x qw per row) so the collective moves big descriptors.
            # flat layout = [feat, q] with feat = 128*n + p.
            rsin = [dram.tile([64, 16 * qw], bf16, name=f"rsin{j}")
                    for j, (_, qw) in enumerate(BLOCKS)]
            rsout = [dram.tile([16, 16 * qw], bf16, name=f"rsout{j}")
                     for j, (_, qw) in enumerate(BLOCKS)]

            # ---- unit builders: each unit is a closure emitting ~1 PE
            # instruction group; the scheduler paces fillers so the PE
            # FIFO never runs dry (idle resets the clock ramp).
            def phase1_units(j):
                """QKV projection for 512-q block j."""
                qs = slice(512 * j, 512 * (j + 1))
                cell = {}
                units = []

                def u_dma():
                    xc = sb.tile([128, LCH, 512], bf16, tag="xc", bufs=2,
                                 name="xc")
                    cell["xc"] = xc
                    xr = x_t[:].rearrange("(l p) s -> p l s", p=128)
                    for l in range(LCH):
                        if j == 0 and l >= LCH // 2:
                            # block 0 is startup-critical: split the load
                            # across two queues
                            nc.gpsimd.dma_start(xc[:, l, :], xr[:, l, qs])
                        else:
                            nc.sync.dma_start(xc[:, l, :], xr[:, l, qs])
                units.append(u_dma)

                def mk_q(c):
                    def u():
                        xc = cell["xc"]
                        qps = ps.tile([128, 512], f32, tag="mm512", bufs=2,
                                      name="qps")
                        for l in range(LCH):
                            nc.tensor.matmul(
                                qps[:], wq_sb[:, l, 128 * c:128 * (c + 1)],
                                xc[:, l, :], start=(l == 0),
                                stop=(l == LCH - 1),
                            )
                        nc.vector.tensor_copy(qT_sb[c][:, qs], qps[:])
                    return u
                units += [mk_q(0), mk_q(1)]

                def u_k():
                    xc = cell["xc"]
                    kps = ps.tile([128, 512], f32, tag="mm512", bufs=2,
                                  name="kps")
                    for l in range(LCH):
                        nc.tensor.matmul(
                            kps[:], wk_sb[:, l, :], xc[:, l, :],
                            start=(l == 0), stop=(l == LCH - 1),
                        )
                    nc.vector.tensor_copy(kT_sb[:, qs], kps[:])
                units.append(u_k)

                def u_v():
                    xc = cell["xc"]
                    vtp = ps.tile([128, 512], f32, tag="mm512", bufs=2,
                                  name="vtp")
                    for l in range(LCH):
                        nc.tensor.matmul(
                            vtp[0:D, :], wv_sb[:, l, :], xc[:, l, :],
                            start=(l == 0), stop=(l == LCH - 1),
                        )
                    vt = sb.tile([64, 512], f32, tag="vt", bufs=2,
                                 name="vt")
                    cell["vt"] = vt
                    nc.vector.tensor_copy(vt[:], vtp[0:D, :])
                units.append(u_v)

                def mk_tr(si):
                    def u():
                        vt = cell["vt"]
                        vps = ps.tile([128, 64], f32, tag="mm512", bufs=2,
                                      name="vps")
                        nc.tensor.transpose(
                            vps[:], vt[:, 128 * si:128 * (si + 1)],
                            eye_sb[:])
                        nc.vector.tensor_copy(
                            vv[:, 4 * j + si, 64:128], vps[:])
                    return u
                units += [mk_tr(si) for si in range(4)]
                if j == 0:
                    # startup: k's weights (gpsimd queue) land before wq's
                    # bulk — run the k chain first while wq streams in
                    units = ([units[0], units[3], units[1], units[2],
                              units[4]] + units[5:])
                return units

            def attention_units(bi, apcs):
                """Attention for block bi; appends normalized attnT pair
                (bf16, [128, qw]) to apcs."""
                q0, qw = BLOCKS[bi]
                qs = slice(q0, q0 + qw)
                nkt = (q0 + qw) // KT
                base = q0 // KT

                def build_p(p):
                    cell = {"pts": {}}

                    def mk_step(t, p=p, cell=cell):
                        def u():
                            pts = cell["pts"]
                            if t < nkt:
                                ks = slice(KT * t, KT * (t + 1))
                                tl = t - base
                                # diagonal strips: columns < 128*tl are
                                # fully masked — skip them in ST/exp/mask
                                # and the trailing OT (~15% of the chain)
                                c0 = max(0, 128 * tl)
                                # head-b always at offset 512 (bank-aligned)
                                st = ps.tile([128, 1024], f32, tag="st",
                                             bufs=2, name="st")
                                nc.tensor.matmul(
                                    st[:, c0:qw], kT_sb[0:64, ks],
                                    qT_sb[p][0:64, q0 + c0:q0 + qw],
                                    start=True, stop=True,
                                    tile_position=(0, 0),
                                )
                                nc.tensor.matmul(
                                    st[:, 512 + c0:512 + qw],
                                    kT_sb[64:128, ks],
                                    qT_sb[p][64:128, q0 + c0:q0 + qw],
                                    start=True, stop=True,
                                    tile_position=(64, 0),
                                )
                                pt = sb.tile([128, 1024], bf16, tag="pt",
                                             bufs=5, name="pt")
                                st2 = st[:].rearrange(
                                    "p (h q) -> p h q", h=2)[:, :, c0:qw]
                                pt2 = pt[:].rearrange(
                                    "p (h q) -> p h q", h=2)[:, :, c0:qw]
                                if tl >= 0:  # diagonal strip: mask after exp
                                    ptr = sb.tile([128, 1024], bf16,
                                                  tag="ptraw", bufs=3,
                                                  name="ptr")
                                    ptr2 = ptr[:].rearrange(
                                        "p (h q) -> p h q", h=2)[:, :, c0:qw]
                                    nc.scalar.activation(
                                        ptr2, st2, Exp, scale=SCALE)
                                    nc.vector.tensor_mul(
                                        pt2, ptr2,
                                        mask_sb[:, 1024 * tl:1024 * (tl + 1)]
                                        .rearrange("p (h q) -> p h q", h=2)
                                        [:, :, c0:qw],
                                    )
                                else:
                                    nc.scalar.activation(
                                        pt2, st2, Exp, scale=SCALE)
                                pts[t] = pt
                            to = t - TRAIL
                            if to >= 0 and to < nkt:
                                ptd = pts.pop(to)
                                first, last = (to == 0), (to == nkt - 1)
                                c0o = max(0, 128 * (to - base))
                                if first:
                                    cell["oa"] = ps.tile(
                                        [128, 512], f32, tag="otrs", bufs=2,
                                        name="oa")
                                    cell["ob"] = ps.tile(
                                        [128, 512], f32, tag="otrs", bufs=2,
                                        name="ob")
                                nc.tensor.matmul(
                                    cell["oa"][:, c0o:qw], vv[:, to, 64:192],
                                    ptd[:, c0o:qw], start=first, stop=last,
                                    skip_group_check=True,
                                )
                                nc.tensor.matmul(
                                    cell["ob"][:, c0o:qw], vv[:, to, 0:128],
                                    ptd[:, 512 + c0o:512 + qw], start=first,
                                    stop=last, skip_group_check=True,
                                )
                        return u
                    p_steps = [mk_step(t) for t in range(nkt + TRAIL)]

                    # Normalize in two stages.  The ones-matmul replicated
                    # the rowsum across 64 partitions; two concurrent
                    # quadrant matmuls against a 1/64 matrix move it to the
                    # opposite halves — cross-partition with NO DMA
                    # (sync-queue DMAs can stall ~15us behind in-flight
                    # collective traffic on the shared rings).
                    def u_norm_a(p=p, cell=cell):
                        oa, ob = cell["oa"], cell["ob"]
                        stg = sb.tile([128, 512], bf16, tag="stg", bufs=2,
                                      name="stg")
                        nc.vector.tensor_copy(stg[64:128, 0:qw],
                                              oa[64:128, 0:qw])
                        nc.vector.tensor_copy(stg[0:64, 0:qw],
                                              ob[0:64, 0:qw])
                        rsm = ps.tile([128, 512], f32, tag="mm512", bufs=2,
                                      name="rsm")
                        cell["rsm"] = rsm
                        nc.tensor.matmul(
                            rsm[0:64, 0:qw], scones[64:128, :],
                            stg[64:128, 0:qw], start=True, stop=True,
                            tile_position=(64, 0),
                        )
                        nc.tensor.matmul(
                            rsm[64:128, 0:qw], scones[0:64, :],
                            stg[0:64, 0:qw], start=True, stop=True,
                            tile_position=(0, 64),
                        )

                    def u_norm_b(p=p, cell=cell):
                        oa, ob = cell["oa"], cell["ob"]
                        rsm = cell["rsm"]
                        rcp = sb.tile([128, 512], f32, tag="rcp", bufs=2,
                                      name="rcp")
                        nc.vector.reciprocal_approx_fast(
                            out=rcp[:, 0:qw], in_=rsm[:, 0:qw])
                        apc = sb.tile([128, 512], bf16, tag="apc", bufs=4,
                                      name="apc")
                        nc.vector.tensor_mul(apc[0:64, 0:qw],
                                             oa[0:64, 0:qw],
                                             rcp[0:64, 0:qw])
                        nc.vector.tensor_mul(apc[64:128, 0:qw],
                                             ob[64:128, 0:qw],
                                             rcp[64:128, 0:qw])
                        apcs.append(apc)

                    return p_steps, u_norm_a, u_norm_b

                s0, na0, nb0 = build_p(0)
                s1, na1, nb1 = build_p(1)
                # p0's normalize stage B lands after 2 of p1's steps so the
                # DVE queue never parks on the reciprocal's wait
                return (s0 + [na0] + s1[:2] + [nb0] + s1[2:]
                        + [na1, nb1])

            def oproj_units(bi, apcs):
                """Row-parallel output projection + ReduceScatter for
                block bi (reads apcs[2*bi:2*bi+2], filled by attention)."""
                q0, qw = BLOCKS[bi]
                cell = {}
                units = []
                # rsin flat = [feat, q], feat = 128*n + p; as a [64, 16*qw]
                # tile, row = feat//16.  AP view [p, n, q]:
                rs_view = rsin[bi][:].rearrange(
                    "(n a) (b s) -> (a b) n s", n=LCH, b=16)

                def mk(n):
                    def u():
                        if n == 0:
                            cell["prt"] = sb.tile([128, LCH, 512], bf16,
                                                  tag="prt", bufs=2,
                                                  name="prt")
                        prt = cell["prt"]
                        a0, a1 = apcs[2 * bi], apcs[2 * bi + 1]
                        pp = ps.tile([128, 512], f32, tag="mm512", bufs=2,
                                     name="pp")
                        nc.tensor.matmul(
                            pp[:, 0:qw], wo_sb[:, 0, 128 * n:128 * (n + 1)],
                            a0[:, 0:qw], start=True, stop=False,
                        )
                        nc.tensor.matmul(
                            pp[:, 0:qw], wo_sb[:, 1, 128 * n:128 * (n + 1)],
                            a1[:, 0:qw], start=False, stop=True,
                        )
                        nc.vector.tensor_copy(prt[:, n, 0:qw], pp[:, 0:qw])
                        if n % 2 == 1:
                            nc.sync.dma_start(
                                rs_view[:, n - 1:n + 1, :],
                                prt[:, n - 1:n + 1, 0:qw])
                    return u
                units += [mk(n) for n in range(LCH)]

                def u_cc():
                    nc.gpsimd.collective_compute(
                        "ReduceScatter", mybir.AluOpType.add,
                        replica_groups=RG,
                        ins=[rsin[bi][:].opt()], outs=[rsout[bi][:].opt()],
                    )
                units.append(u_cc)
                return units

            def mk_copy(bi):
                """out-copy for block bi.  MUST be scheduled well after
                RS(bi) completes: a DMA whose wait is still unsatisfied
                parks its whole (in-order) engine queue, and its
                descriptors straddle the collective-owned rings."""
                q0, qw = BLOCKS[bi]

                def u():
                    nc.sync.dma_start(
                        out[:, q0:q0 + qw],
                        rsout[bi][:].rearrange("x (y s) -> (x y) s", y=16))
                return u

            def run_units(units):
                for u in units:
                    u()

            def interleave(main_units, filler_units, delay=0, frac=0.65):
                """Run main units; pull filler units paced so they finish
                by `frac` of the mains (front-loaded so ReduceScatter
                inputs land early). `delay` skips fillers for the first
                few main units."""
                m = len(main_units)
                f = len(filler_units)
                span = max(1, int((m - delay) * frac))
                fi = 0
                for i, u in enumerate(main_units):
                    u()
                    if i < delay:
                        continue
                    while fi < f and (fi + 1) * span <= (i + 1 - delay) * f:
                        filler_units[fi]()
                        fi += 1
                while fi < f:
                    filler_units[fi]()
                    fi += 1

            apcs = []
            run_units(phase1_units(0))
            interleave(attention_units(0, apcs), phase1_units(1))
            interleave(attention_units(1, apcs),
                       oproj_units(0, apcs) + phase1_units(2), delay=3)
            interleave(attention_units(2, apcs),
                       oproj_units(1, apcs) + phase1_units(3), delay=3)
            interleave(attention_units(3, apcs),
                       oproj_units(2, apcs), delay=3)
            interleave(attention_units(4, apcs),
                       oproj_units(3, apcs), delay=3)
            run_units(oproj_units(4, apcs))
            # all out-copies at the very end: a copy emitted mid-stream
            # parks its queue (and its DMA ring slots) on a possibly-late
            # collective; here every RS but the last is long done
            run_units([mk_copy(bi) for bi in range(len(BLOCKS))])

    nc.finalize()
    return nc


def _shard_inputs(x, w_qkv, w_out):
    """Build the per-core input maps (host-side sharding only)."""
    x = np.asarray(x, dtype=np.float32)
    w_qkv = np.asarray(w_qkv, dtype=np.float32)
    w_out = np.asarray(w_out, dtype=np.float32)
    bf = ml_dtypes.bfloat16

    # causal masks for the 4 diagonal k-tile offsets, replicated for the
    # two heads packed side by side in each 1024-wide strip
    kk = np.arange(128)[:, None]
    qq = np.arange(512)[None, :]
    strips = []
    for t in range(4):
        m = (kk <= qq - 128 * t).astype(np.float32)  # [128, 512]
        strips.append(np.concatenate([m, m], axis=1))  # [128, 1024]
    mask = np.ascontiguousarray(np.concatenate(strips, axis=1))  # [128, 4096]

    def pack(w_t, inner):
        """[LATENT, inner] -> SBUF layout [128, LCH*inner] (row-major
        (l, m) per partition)."""
        r = w_t.reshape(LCH, 128, inner).transpose(1, 0, 2)
        return np.ascontiguousarray(r.reshape(128, LCH * inner))

    in_maps = []
    for c in range(NCORES):
        b, r = divmod(c, TP)
        wq = w_qkv[DQ * r:DQ * (r + 1), :]                    # [256, 1024]
        wk = w_qkv[H * D + D * r:H * D + D * (r + 1), :]      # [64, 1024]
        wv = w_qkv[(H + HK) * D + D * r:(H + HK) * D + D * (r + 1), :]
        wo = w_out[:, DQ * r:DQ * (r + 1)]                    # [1024, 256]
        wo_p = np.ascontiguousarray(wo.T).reshape(2, 128, LATENT)
        wo_p = np.ascontiguousarray(wo_p.transpose(1, 0, 2)
                                    .reshape(128, 2 * LATENT))
        in_maps.append({
            "eye": np.eye(64, dtype=np.float32),
            "x_t": np.ascontiguousarray(x[b].T).astype(bf),
            "wq_t": pack(np.ascontiguousarray(wq.T), DQ).astype(bf),
            "wk_d": pack(np.ascontiguousarray(
                np.concatenate([wk.T, wk.T], axis=1)), 128).astype(bf),
            "wv_t": pack(np.ascontiguousarray(wv.T), D).astype(bf),
            "wo_t": wo_p.astype(bf),
            "mask": mask.astype(bf),
        })
    return in_maps


def _get_nc():
    if "nc" not in _CACHE:
        _CACHE["nc"] = _build()
    return _CACHE["nc"]


def _install_ntff_shim():
    """Make BASS_TRACE work under axon (antenv.axon_hooks is absent here)."""
    import types
    if "antenv.axon_hooks" in sys.modules:
        return True
    try:
        import antenv
        from trn_agent_boot.trn_boot import _ntff_profile_via_ctypes
        hook = _ntff_profile_via_ctypes("/opt/axon/libaxon_pjrt.so")
        if hook is None:
            return False
        mod = types.ModuleType("antenv.axon_hooks")
        state = {"hook": hook}
        mod.set_axon_ntff_profile_hook = lambda h: state.__setitem__("hook", h)
        mod.get_axon_ntff_profile_hook = lambda: state["hook"]
        sys.modules["antenv.axon_hooks"] = mod
        antenv.axon_hooks = mod
        return True
    except Exception:
        return False


LAST_RESULT = None


def kernel(x, w_qkv, w_out):
    global LAST_RESULT
    from concourse.bass_utils import run_bass_kernel_spmd

    nc = _get_nc()
    in_maps = _shard_inputs(x, w_qkv, w_out)

    trace = bool(os.environ.get("BASS_TRACE"))
    if trace:
        trace = _install_ntff_shim()
    kwargs = {}
    if trace and os.environ.get("BASS_TRACE_CORES") == "all":
        kwargs["trace_cores"] = list(range(NCORES))
    res = run_bass_kernel_spmd(
        nc, in_maps, core_ids=list(range(NCORES)), trace=trace, **kwargs
    )
    LAST_RESULT = res

    full = np.empty((B, S, LATENT), dtype=np.float32)
    for c in range(NCORES):
        b, r = divmod(c, TP)
        full[b, :, DQ * r:DQ * (r + 1)] = np.asarray(
            res.results[c]["out"], dtype=np.float32).T
    return full
